# revision 5
# baseline (speedup 1.0000x reference)
"""Trainium2 Bass kernel for nn_Listener (LSTM listener + dense encoders).

Reference computation (per full batch B=512):
    emb = embed_table[message]                       # [B, T, 512]
    LSTM over T=128 steps, HIDDEN=1024:
        gated = [x_t, h] @ W_cell + b_cell           # [B, 4096] (i, g, f, o)
        f = sigmoid(f + 1); c = f*c + sigmoid(i)*tanh(g); h = sigmoid(o)*tanh(c)
    images_encoded = images @ W_img + b_img          # [B, 1024]
    hidden_encoded = h @ W_hid + b_hid               # [B, 1024]
    returns (images_encoded, hidden_encoded)

Strategy (8 NeuronCores, data-parallel over batch, 64 rows/core):
  * Embedding lookup + x-projection fold into one table:
        M2 = embed_table @ W_cell[:512] + b_cell  (+1 on the f columns)
    stored bf16, gathered per step by token id with PARTITION layout:
    gathered row p holds (batch p, unit-half 0) for p<64 and
    (batch p-64, unit-half 1) for p>=64 -- aligned with the packed PSUM
    partition layout, so x enters the gates either by an IDENTITY
    selection matmul (f/o gates, PE) or by an in-place DVE add onto the
    stopped PSUM bank (g/i gates) -- removing ~0.5us/step of PE inject.
  * Per-core batch is 64 = half the PE output partitions; hidden units
    split in half across PSUM partition ranges; the two column-groups of
    the PE run concurrently (216 ns per N=512 pair, measured).
  * Stream order (g, i1, i2, f1, f2, o1, o2); g/i banks self-seed with
    start=True on their first pair; i split into two banks so its x-add
    starts at the half-stream stop and sigmoid(i) lands early enough to
    keep the c -> tanh(c) chain off the step-boundary critical path.
  * Epilogue ACT order tg, si1, si2, sf1, sf2, tc1, so1, so2, tc2 so
    h-half0 = sigmoid(o1)*tanh(c1) is ready when the o2 stream ends; the
    PE then runs tp q0,q1 + f/o injects for t+1 back-to-back and the
    first pairs of t+1 start ~0.6us after the stream -- the tp q2,q3 +
    final cast ride inside the next stream.
  * Weight DMA order: small consts + images first (PE warmup
    transposes), then W_h (needed by step 1), encoder weights last.
"""

import os
import numpy as np

B, T = 512, 128
HIDDEN = 1024
VOCAB = 1024
EMB = 512
OUT = 1024
D_IMG = 2048
NCORES = 8
BS = B // NCORES  # 64 batch rows per core
HH = HIDDEN // 2  # 512 = per-half hidden units
HQ = HH // 2      # 256

_CACHE = {}

# gate order inside the packed tables / PSUM banks: (g, i, f, o)
GATE_PERM = [1, 0, 2, 3]  # indices into natural (i, g, f, o)
CI_ORDER = [0, 4, 1, 5, 2, 6, 3, 7]  # k-chunks: first 4 need tp blocks 0,1

GRID = 0.025  # ms of sim-time per step slot (floors are ordering only)


def _build_nc(n_steps: int):
    import concourse.bass as bass
    import concourse.mybir as mybir
    from concourse import bacc, tile

    f32 = mybir.dt.float32
    f32r = mybir.dt.float32r
    bf16 = mybir.dt.bfloat16
    i32 = mybir.dt.int32
    AF = mybir.ActivationFunctionType

    nc = bacc.Bacc("TRN2", target_bir_lowering=False, debug=False)

    m2p_d = nc.declare_dram_parameter("m2p", [2 * VOCAB, HH * 4], bf16, isOutput=False)
    wh_d = nc.declare_dram_parameter("wh", [HIDDEN, 4 * HIDDEN], bf16, isOutput=False)
    msg2_d = nc.declare_dram_parameter("msg2", [2 * BS, T], i32, isOutput=False)
    identb_d = nc.declare_dram_parameter("identb", [128, 128], bf16, isOutput=False)
    imgs_d = nc.declare_dram_parameter("imgs", [128, D_IMG // 2], f32, isOutput=False)
    wimg_d = nc.declare_dram_parameter("wimg", [D_IMG, OUT], bf16, isOutput=False)
    whid_d = nc.declare_dram_parameter("whid", [HIDDEN, OUT], bf16, isOutput=False)
    o2_d = nc.declare_dram_parameter("o2", [2, 128], f32r, isOutput=False)
    bimg2_d = nc.declare_dram_parameter("bimg2", [2, OUT // 2], f32r, isOutput=False)
    bhid2_d = nc.declare_dram_parameter("bhid2", [2, OUT // 2], f32r, isOutput=False)
    oimg_d = nc.declare_dram_parameter("oimg", [128, OUT // 2], f32, isOutput=True)
    ohid_d = nc.declare_dram_parameter("ohid", [128, OUT // 2], f32, isOutput=True)

    with tile.TileContext(nc) as tc:
        with (
            tc.tile_pool(name="wpool", bufs=1) as wpool,
            tc.tile_pool(name="const", bufs=1) as cpool,
            tc.tile_pool(name="xg", bufs=3) as xgpool,
            tc.tile_pool(name="state", bufs=2) as stpool,
            tc.tile_pool(name="act", bufs=1) as apool,
            tc.tile_pool(name="outs", bufs=1) as opool,
            tc.tile_pool(name="psum", bufs=1, space="PSUM") as pspool,
        ):
            # ---- small constants first (cheap, needed early) ----
            msg2 = cpool.tile([2 * BS, T], i32, tag="msg2")
            nc.sync.dma_start(msg2[:], msg2_d[:])
            identb = cpool.tile([128, 128], bf16, tag="identb")
            nc.sync.dma_start(identb[:], identb_d[:])
            o2 = cpool.tile([2, 128], f32r, tag="o2")
            nc.sync.dma_start(o2[:], o2_d[:])
            bimg2 = cpool.tile([2, OUT // 2], f32r, tag="bimg2")
            nc.sync.dma_start(bimg2[:], bimg2_d[:])
            bhid2 = cpool.tile([2, OUT // 2], f32r, tag="bhid2")
            nc.sync.dma_start(bhid2[:], bhid2_d[:])

            # ---- images next (PE warmup transposes while W_h streams in) ----
            # imgs packed [128, 1024]: partitions 0:64 = batch x feats 0:1024,
            # partitions 64:128 = batch x feats 1024:2048.
            imgs = cpool.tile([128, D_IMG // 2], f32, tag="imgs")
            nc.sync.dma_start(imgs[:], imgs_d[:])
            imgsb = cpool.tile([128, D_IMG // 2], bf16, tag="imgsb")
            nc.vector.tensor_copy(imgsb[:], imgs[:])
            imT = cpool.tile([128, D_IMG // 2], bf16, tag="imT")
            for half in range(2):
                tpw = pspool.tile([128, 8 * BS], bf16, tag="tp", name=f"tpi{half}")
                for q in range(4):
                    qq = 4 * half + q
                    nc.tensor.transpose(
                        out=tpw[:, 128 * q : 128 * (q + 1)],
                        in_=imgsb[:, 128 * qq : 128 * (qq + 1)],
                        identity=identb[:],
                    )
                nc.vector.tensor_copy(imT[:, 512 * half : 512 * (half + 1)], tpw[:])

            # ---- W_h resident in SBUF: 8 chunks of [128, 4096] ----
            # (issued before the encoder weights -- step 1 gates on these)
            wh_sb = []
            for ci in range(8):
                wt = wpool.tile([128, 4 * HIDDEN], bf16, tag=f"wh{ci}")
                nc.sync.dma_start(wt[:], wh_d[128 * ci : 128 * (ci + 1), :])
                wh_sb.append(wt)
            # encoder weights last (only needed at the end; DMA overlaps loop)
            whid_sb = []
            for ci in range(8):
                wt = wpool.tile([128, OUT], bf16, tag=f"whid{ci}")
                nc.sync.dma_start(wt[:], whid_d[128 * ci : 128 * (ci + 1), :])
                whid_sb.append(wt)
            wimg_sb = []
            for ci in range(16):
                wt = wpool.tile([128, OUT], bf16, tag=f"wimg{ci}")
                nc.sync.dma_start(wt[:], wimg_d[128 * ci : 128 * (ci + 1), :])
                wimg_sb.append(wt)

            def hT_sl(hT, ci):
                # packed-transpose layout: block q holds chunk q (cols 0:64)
                # and chunk q+4 (cols 64:128) at col block 128*q
                q, hi = (ci - 4, 64) if ci >= 4 else (ci, 0)
                return hT[:, 128 * q + hi : 128 * q + hi + 64]

            def imT_sl(ci):
                q, hi = (ci - 8, 64) if ci >= 8 else (ci, 0)
                return imT[:, 128 * q + hi : 128 * q + hi + 64]

            def gather(t):
                xg = xgpool.tile([2 * BS, 4 * HH], bf16, tag="xg", name=f"xg_{t}")
                nc.gpsimd.indirect_dma_start(
                    out=xg[:],
                    out_offset=None,
                    in_=m2p_d[:],
                    in_offset=bass.IndirectOffsetOnAxis(ap=msg2[:, t : t + 1], axis=0),
                )
                return xg

            # PSUM banks per step: g [512], i1, i2, f1, f2, o1, o2 (cols 0:256
            # used in the split banks) + one shared tp tile = 8 banks.
            BANK_KEYS = ("g", "i1", "i2", "f1", "f2", "o1", "o2")

            def alloc_banks(t):
                return {
                    k: pspool.tile([128, HH], f32, tag=f"gp_{k}", name=f"gp_{k}_{t}")
                    for k in BANK_KEYS
                }

            # stream spec: (key, gate, col0, col1) in packed-gate columns
            STREAMS = (
                ("g", 0, 0, HH),
                ("i1", 1, 0, HQ),
                ("i2", 1, HQ, HH),
                ("f1", 2, 0, HQ),
                ("f2", 2, HQ, HH),
                ("o1", 3, 0, HQ),
                ("o2", 3, HQ, HH),
            )
            PE_FLOORS = {
                "g": 0.0000, "i1": 0.0013, "i2": 0.0016,
                "f1": 0.0019, "f2": 0.0022, "o1": 0.0025, "o2": 0.0028,
            }

            def inject_id(bank, xg, gc0, gc1, base_off):
                # identity selection: bank[:, 0:(gc1-gc0)] = xg[:, gc0:gc1]
                with tc.tile_wait_until(base_off):
                    nc.tensor.matmul(
                        out=bank[:, 0 : gc1 - gc0],
                        lhsT=identb[:],
                        rhs=xg[:, gc0:gc1],
                        start=True,
                        stop=False,
                        skip_group_check=True,
                    )

            # ---- prologue: gathers for steps 0..2 ----
            xg_t = [None] * (n_steps + 2)
            xg_t[0] = gather(0)
            xg_t[1] = gather(1)
            xg_t[2] = gather(2)

            # ---- step 0 epilogue (h0 = 0: gates come from xg only) ----
            base = 0.0
            tg = apool.tile([128, HH], bf16, tag="tg", name="tg_0")
            si = apool.tile([128, HH], bf16, tag="si", name="si_0")
            with tc.tile_wait_until(base + 0.0050):
                nc.scalar.activation(tg[:], xg_t[0][:, 0:HH], AF.Tanh)
            with tc.tile_wait_until(base + 0.0068):
                nc.scalar.activation(si[:], xg_t[0][:, HH : 2 * HH], AF.Sigmoid)
            m1a = apool.tile([128, HQ], bf16, tag="m1a", name="m1a_0")
            m1b = apool.tile([128, HQ], bf16, tag="m1b", name="m1b_0")
            with tc.tile_wait_until(base + 0.0075):
                nc.vector.tensor_mul(m1a[:], si[:, 0:HQ], tg[:, 0:HQ])
            with tc.tile_wait_until(base + 0.0091):
                nc.vector.tensor_mul(m1b[:], si[:, HQ:HH], tg[:, HQ:HH])
            cA = stpool.tile([128, HQ], f32, tag="cA", name="cA_0")
            cB = stpool.tile([128, HQ], f32, tag="cB", name="cB_0")
            with tc.tile_wait_until(base + 0.0104):
                nc.vector.tensor_copy(cA[:], m1a[:])
            with tc.tile_wait_until(base + 0.0120):
                nc.vector.tensor_copy(cB[:], m1b[:])
            tc1 = apool.tile([128, HQ], bf16, tag="tc1", name="tc1_0")
            tc2 = apool.tile([128, HQ], bf16, tag="tc2", name="tc2_0")
            so1 = apool.tile([128, HQ], bf16, tag="so1", name="so1_0")
            so2 = apool.tile([128, HQ], bf16, tag="so2", name="so2_0")
            with tc.tile_wait_until(base + 0.0114):
                nc.scalar.activation(tc1[:], cA[:], AF.Tanh)
            with tc.tile_wait_until(base + 0.0128):
                nc.scalar.activation(so1[:], xg_t[0][:, 3 * HH : 3 * HH + HQ], AF.Sigmoid)
            with tc.tile_wait_until(base + 0.0145):
                nc.scalar.activation(so2[:], xg_t[0][:, 3 * HH + HQ : 4 * HH], AF.Sigmoid)
            with tc.tile_wait_until(base + 0.0131):
                nc.scalar.activation(tc2[:], cB[:], AF.Tanh)
            h1 = apool.tile([128, HQ], bf16, tag="h1", name="h1_0")
            h2 = apool.tile([128, HQ], bf16, tag="h2", name="h2_0")
            with tc.tile_wait_until(base + 0.0133):
                nc.vector.tensor_mul(h1[:], so1[:], tc1[:])
            with tc.tile_wait_until(base + 0.0150):
                nc.vector.tensor_mul(h2[:], so2[:], tc2[:])

            # transposes + casts for step 0 (tp blocks 0,1 then 2,3)
            tp = pspool.tile([128, 8 * BS], bf16, tag="tp", name="tp_0")
            hT_cur = stpool.tile([128, 8 * BS], bf16, tag="hT", name="hT_0")
            with tc.tile_wait_until(base + 0.0142):
                for q in range(2):
                    nc.tensor.transpose(
                        out=tp[:, 128 * q : 128 * (q + 1)],
                        in_=h1[:, 128 * q : 128 * (q + 1)],
                        identity=identb[:],
                    )
            with tc.tile_wait_until(base + 0.0146):
                nc.vector.tensor_copy(hT_cur[:, 0:128], tp[:, 0:128])
            with tc.tile_wait_until(base + 0.0148):
                nc.vector.tensor_copy(hT_cur[:, 128:256], tp[:, 128:256])
            # f/o injects for step 1 + first-half transposes done; the rest
            # (tp q2,q3 + cast + o2 inject) floored into slot 1.
            gpb_cur = alloc_banks(1)
            inject_id(gpb_cur["f1"], xg_t[1], 2 * HH, 2 * HH + HQ, base + 0.0150)
            inject_id(gpb_cur["f2"], xg_t[1], 2 * HH + HQ, 3 * HH, base + 0.0152)
            inject_id(gpb_cur["o1"], xg_t[1], 3 * HH, 3 * HH + HQ, base + 0.0154)
            inject_id(gpb_cur["o2"], xg_t[1], 3 * HH + HQ, 4 * HH, base + 0.0156)
            with tc.tile_wait_until(GRID + 0.0005):
                for q in range(2, 4):
                    nc.tensor.transpose(
                        out=tp[:, 128 * q : 128 * (q + 1)],
                        in_=h2[:, 128 * (q - 2) : 128 * (q - 1)],
                        identity=identb[:],
                    )
            with tc.tile_wait_until(GRID + 0.0007):
                nc.vector.tensor_copy(hT_cur[:, 256:512], tp[:, 256:512])

            cA_prev, cB_prev = cA, cB

            # ---- recurrence ----
            for t in range(1, n_steps):
                last_t = t == n_steps - 1
                base = GRID * t
                if t + 2 <= n_steps - 1:
                    xg_t[t + 2] = gather(t + 2)
                xg = xg_t[t]
                gpb = gpb_cur

                # h @ W_h pair streams
                for k, gate, c0, c1 in STREAMS:
                    w0 = 1024 * gate
                    self_seed = k in ("g", "i1", "i2")
                    with tc.tile_wait_until(base + PE_FLOORS[k]):
                        for idx, ci in enumerate(CI_ORDER):
                            first = idx == 0
                            last = idx == len(CI_ORDER) - 1
                            lhs = hT_sl(hT_cur, ci)
                            oc = c1 - c0 if k == "g" else HQ
                            nc.tensor.matmul(
                                out=gpb[k][0:64, 0:oc],
                                lhsT=lhs,
                                rhs=wh_sb[ci][:, w0 + c0 : w0 + c1],
                                start=first and self_seed,
                                stop=last,
                                skip_group_check=True,
                            )
                            nc.tensor.matmul(
                                out=gpb[k][64:128, 0:oc],
                                lhsT=lhs,
                                rhs=wh_sb[ci][:, w0 + 512 + c0 : w0 + 512 + c1],
                                start=first and self_seed,
                                stop=last,
                                skip_group_check=True,
                            )
                            if k == "g" and idx == 0:
                                # leave the sub-floor window for o2-inject +
                                # tp q2,q3 of the previous step right after
                                # the first pair
                                tc.tile_set_cur_wait(base + 0.0010)

                # ---- x adds for g/i (in-place on stopped PSUM banks) ----
                with tc.tile_wait_until(base + 0.0040):
                    nc.vector.tensor_add(gpb["g"][:, 0:HH], gpb["g"][:, 0:HH],
                                         xg[:, 0:HH])
                with tc.tile_wait_until(base + 0.0056):
                    nc.vector.tensor_add(gpb["i1"][:, 0:HQ], gpb["i1"][:, 0:HQ],
                                         xg[:, HH : HH + HQ])
                with tc.tile_wait_until(base + 0.0073):
                    nc.vector.tensor_add(gpb["i2"][:, 0:HQ], gpb["i2"][:, 0:HQ],
                                         xg[:, HH + HQ : 2 * HH])

                # ---- epilogue: gates -> c, h ----
                tg = apool.tile([128, HH], bf16, tag="tg", name=f"tg_{t}")
                si1 = apool.tile([128, HQ], bf16, tag="si1", name=f"si1_{t}")
                si2 = apool.tile([128, HQ], bf16, tag="si2", name=f"si2_{t}")
                with tc.tile_wait_until(base + 0.0050):
                    nc.scalar.activation(tg[:], gpb["g"][:, 0:HH], AF.Tanh)
                with tc.tile_wait_until(base + 0.0068):
                    nc.scalar.activation(si1[:], gpb["i1"][:, 0:HQ], AF.Sigmoid)
                with tc.tile_wait_until(base + 0.0085):
                    nc.scalar.activation(si2[:], gpb["i2"][:, 0:HQ], AF.Sigmoid)
                m1a = apool.tile([128, HQ], bf16, tag="m1a", name=f"m1a_{t}")
                m1b = apool.tile([128, HQ], bf16, tag="m1b", name=f"m1b_{t}")
                with tc.tile_wait_until(base + 0.0075):
                    nc.vector.tensor_mul(m1a[:], si1[:], tg[:, 0:HQ])
                with tc.tile_wait_until(base + 0.0091):
                    nc.vector.tensor_mul(m1b[:], si2[:], tg[:, HQ:HH])
                sf1 = apool.tile([128, HQ], bf16, tag="sf1", name=f"sf1_{t}")
                sf2 = apool.tile([128, HQ], bf16, tag="sf2", name=f"sf2_{t}")
                with tc.tile_wait_until(base + 0.0094):
                    nc.scalar.activation(sf1[:], gpb["f1"][:, 0:HQ], AF.Sigmoid)
                with tc.tile_wait_until(base + 0.0110):
                    nc.scalar.activation(sf2[:], gpb["f2"][:, 0:HQ], AF.Sigmoid)
                cm1 = apool.tile([128, HQ], f32, tag="cm1", name=f"cm1_{t}")
                cm2 = apool.tile([128, HQ], f32, tag="cm2", name=f"cm2_{t}")
                cA = stpool.tile([128, HQ], f32, tag="cA", name=f"cA_{t}")
                cB = stpool.tile([128, HQ], f32, tag="cB", name=f"cB_{t}")
                with tc.tile_wait_until(base + 0.0100):
                    nc.vector.tensor_mul(cm1[:], sf1[:], cA_prev[:])
                with tc.tile_wait_until(base + 0.0104):
                    nc.vector.tensor_add(cA[:], cm1[:], m1a[:])
                with tc.tile_wait_until(base + 0.0116):
                    nc.vector.tensor_mul(cm2[:], sf2[:], cB_prev[:])
                with tc.tile_wait_until(base + 0.0120):
                    nc.vector.tensor_add(cB[:], cm2[:], m1b[:])
                tc1 = apool.tile([128, HQ], bf16, tag="tc1", name=f"tc1_{t}")
                tc2 = apool.tile([128, HQ], bf16, tag="tc2", name=f"tc2_{t}")
                so1 = apool.tile([128, HQ], bf16, tag="so1", name=f"so1_{t}")
                so2 = apool.tile([128, HQ], bf16, tag="so2", name=f"so2_{t}")
                with tc.tile_wait_until(base + 0.0114):
                    nc.scalar.activation(tc1[:], cA[:], AF.Tanh)
                with tc.tile_wait_until(base + 0.0128):
                    nc.scalar.activation(so1[:], gpb["o1"][:, 0:HQ], AF.Sigmoid)
                with tc.tile_wait_until(base + 0.0145):
                    nc.scalar.activation(so2[:], gpb["o2"][:, 0:HQ], AF.Sigmoid)
                with tc.tile_wait_until(base + 0.0131):
                    nc.scalar.activation(tc2[:], cB[:], AF.Tanh)
                h1 = apool.tile([128, HQ], bf16, tag="h1", name=f"h1_{t}")
                h2 = apool.tile([128, HQ], bf16, tag="h2", name=f"h2_{t}")
                with tc.tile_wait_until(base + 0.0133):
                    nc.vector.tensor_mul(h1[:], so1[:], tc1[:])
                with tc.tile_wait_until(base + 0.0150):
                    nc.vector.tensor_mul(h2[:], so2[:], tc2[:])

                # ---- h -> hT: tp q0,q1 + casts now; q2,q3 + cast in next slot
                tp = pspool.tile([128, 8 * BS], bf16, tag="tp", name=f"tp_{t}")
                hT_next = stpool.tile([128, 8 * BS], bf16, tag="hT", name=f"hT_{t}")
                with tc.tile_wait_until(base + 0.0142):
                    for q in range(2):
                        nc.tensor.transpose(
                            out=tp[:, 128 * q : 128 * (q + 1)],
                            in_=h1[:, 128 * q : 128 * (q + 1)],
                            identity=identb[:],
                        )
                with tc.tile_wait_until(base + 0.0146):
                    nc.vector.tensor_copy(hT_next[:, 0:128], tp[:, 0:128])
                with tc.tile_wait_until(base + 0.0148):
                    nc.vector.tensor_copy(hT_next[:, 128:256], tp[:, 128:256])

                if not last_t:
                    gpb_next = alloc_banks(t + 1)
                    inject_id(gpb_next["f1"], xg_t[t + 1], 2 * HH, 2 * HH + HQ,
                              base + 0.0150)
                    inject_id(gpb_next["f2"], xg_t[t + 1], 2 * HH + HQ, 3 * HH,
                              base + 0.0152)
                    inject_id(gpb_next["o1"], xg_t[t + 1], 3 * HH, 3 * HH + HQ,
                              base + 0.0154)
                    inject_id(gpb_next["o2"], xg_t[t + 1], 3 * HH + HQ, 4 * HH,
                              base + 0.0156)
                with tc.tile_wait_until(base + GRID + 0.0005):
                    for q in range(2, 4):
                        nc.tensor.transpose(
                            out=tp[:, 128 * q : 128 * (q + 1)],
                            in_=h2[:, 128 * (q - 2) : 128 * (q - 1)],
                            identity=identb[:],
                        )
                with tc.tile_wait_until(base + GRID + 0.0007):
                    nc.vector.tensor_copy(hT_next[:, 256:512], tp[:, 256:512])

                cA_prev, cB_prev = cA, cB
                hT_cur = hT_next
                if not last_t:
                    gpb_cur = gpb_next

            # ---- hidden encoder: out = h @ W_hid + b_hid ----
            ohp = pspool.tile([128, OUT // 2], f32, tag="gp_g", name="enc_hid")
            nc.tensor.matmul(
                out=ohp[:], lhsT=o2[:], rhs=bhid2[:],
                start=True, stop=False, skip_group_check=True,
            )
            for ci in range(8):
                last = ci == 7
                lhs = hT_sl(hT_cur, ci)
                nc.tensor.matmul(
                    out=ohp[0:64, :], lhsT=lhs, rhs=whid_sb[ci][:, 0:512],
                    start=False, stop=last, skip_group_check=True,
                )
                nc.tensor.matmul(
                    out=ohp[64:128, :], lhsT=lhs, rhs=whid_sb[ci][:, 512:1024],
                    start=False, stop=last, skip_group_check=True,
                )
            ohid_sb = opool.tile([128, OUT // 2], f32, tag="ohid")
            nc.vector.tensor_copy(ohid_sb[:], ohp[:])
            nc.sync.dma_start(ohid_d[:], ohid_sb[:])

            # ---- images encoder: out = images @ W_img + b_img ----
            oip = pspool.tile([128, OUT // 2], f32, tag="gp_i1", name="enc_img")
            nc.tensor.matmul(
                out=oip[:], lhsT=o2[:], rhs=bimg2[:],
                start=True, stop=False, skip_group_check=True,
            )
            for ci in range(16):
                last = ci == 15
                lhs = imT_sl(ci)
                nc.tensor.matmul(
                    out=oip[0:64, :], lhsT=lhs, rhs=wimg_sb[ci][:, 0:512],
                    start=False, stop=last, skip_group_check=True,
                )
                nc.tensor.matmul(
                    out=oip[64:128, :], lhsT=lhs, rhs=wimg_sb[ci][:, 512:1024],
                    start=False, stop=last, skip_group_check=True,
                )
            oimg_sb = opool.tile([128, OUT // 2], f32, tag="oimg")
            nc.vector.tensor_copy(oimg_sb[:], oip[:])
            nc.sync.dma_start(oimg_d[:], oimg_sb[:])

    nc.compile()
    return nc


def _host_prep(images, embed_table, W_cell, b_cell, W_img, b_img, W_hid, b_hid,
               message):
    """Builds the per-core input maps (all host-side preprocessing)."""
    from ml_dtypes import bfloat16

    W_x = W_cell[:EMB]          # [512, 4096]
    W_h = W_cell[EMB:]          # [1024, 4096]

    M2 = embed_table.astype(np.float32) @ W_x + b_cell  # [1024, 4096]
    M2[:, 2 * HIDDEN : 3 * HIDDEN] += 1.0  # fold the f-gate +1.0
    # permute gate blocks to bank order (g, i, f, o)
    M2 = np.concatenate(
        [M2[:, 1024 * p : 1024 * (p + 1)] for p in GATE_PERM], axis=1
    )
    W_hp = np.concatenate(
        [W_h[:, 1024 * p : 1024 * (p + 1)] for p in GATE_PERM], axis=1
    ).astype(bfloat16)
    # row 2v+h = [g_h, i_h, f_h, o_h] halves of vocab row v
    M2p = np.ascontiguousarray(
        M2.reshape(VOCAB, 4, 2, HH).transpose(0, 2, 1, 3).reshape(2 * VOCAB, 4 * HH)
    ).astype(bfloat16)

    ident = np.eye(128, dtype=np.float32)
    identb = ident.astype(bfloat16)

    o2 = np.zeros((2, 128), np.float32)
    o2[0, 0:64] = 1.0
    o2[1, 64:128] = 1.0

    W_img_b = W_img.astype(bfloat16)
    W_hid_b = W_hid.astype(bfloat16)
    bimg2 = np.stack([b_img[: OUT // 2], b_img[OUT // 2 :]]).astype(np.float32)
    bhid2 = np.stack([b_hid[: OUT // 2], b_hid[OUT // 2 :]]).astype(np.float32)

    in_maps = []
    for core in range(NCORES):
        sl = slice(core * BS, (core + 1) * BS)
        msg = message[sl]  # [64, T] int32
        # partition layout: row p (p<64) -> half0 of batch p; row 64+p -> half1
        msg2 = np.empty((2 * BS, T), np.int32)
        msg2[0:BS] = 2 * msg
        msg2[BS:] = 2 * msg + 1
        in_maps.append(
            {
                "m2p": M2p,
                "wh": W_hp,
                "msg2": msg2,
                "identb": identb,
                "imgs": np.concatenate(
                    [images[sl, : D_IMG // 2], images[sl, D_IMG // 2 :]], axis=0
                ),
                "wimg": W_img_b,
                "whid": W_hid_b,
                "o2": o2,
                "bimg2": bimg2,
                "bhid2": bhid2,
            }
        )
    return in_maps


def kernel(images, embed_table, W_cell, b_cell, W_img, b_img, W_hid, b_hid,
           message):
    import sys
    if "/opt/trn_rl_repo" not in sys.path:
        sys.path.insert(0, "/opt/trn_rl_repo")
    from concourse.bass_utils import run_bass_kernel_spmd

    images = np.asarray(images, np.float32)
    embed_table = np.asarray(embed_table, np.float32)
    W_cell = np.asarray(W_cell, np.float32)
    b_cell = np.asarray(b_cell, np.float32)
    W_img = np.asarray(W_img, np.float32)
    b_img = np.asarray(b_img, np.float32)
    W_hid = np.asarray(W_hid, np.float32)
    b_hid = np.asarray(b_hid, np.float32)
    message = np.asarray(message, np.int32)

    n_steps = T
    if "nc" not in _CACHE or _CACHE.get("n_steps") != n_steps:
        _CACHE["nc"] = _build_nc(n_steps)
        _CACHE["n_steps"] = n_steps
    nc = _CACHE["nc"]

    in_maps = _host_prep(
        images, embed_table, W_cell, b_cell, W_img, b_img, W_hid, b_hid, message
    )
    res = run_bass_kernel_spmd(nc, in_maps, core_ids=list(range(NCORES)))
    results = res.results

    images_encoded = np.empty((B, OUT), np.float32)
    hidden_encoded = np.empty((B, OUT), np.float32)
    for core in range(NCORES):
        sl = slice(core * BS, (core + 1) * BS)
        oi = results[core]["oimg"]
        oh = results[core]["ohid"]
        images_encoded[sl, : OUT // 2] = oi[0:64]
        images_encoded[sl, OUT // 2 :] = oi[64:128]
        hidden_encoded[sl, : OUT // 2] = oh[0:64]
        hidden_encoded[sl, OUT // 2 :] = oh[64:128]
    return images_encoded, hidden_encoded
